# revision 25
# baseline (speedup 1.0000x reference)
"""Trainium2 Bass kernel for nn_Decouple (per-pixel dynamic 3x3 kernel with
dilation 2, then 3x3 conv + bias + LeakyReLU 0.2).

Sharding: pure data parallel over 8 cores; core c handles image n = c//2,
output rows [96*s, 96*s+96) with s = c%2. Inside each core the 96 rows are
split into two 48-row halves stacked on the 128 SBUF partitions
(partitions 0-63 = half A channels, 64-127 = half B channels).

y and x are cast to bf16 on the host (halves HBM traffic; rel err ~5e-3,
gate is 2e-2). y is pre-arranged on the host into per-(tile, tap-group,
half) blocks that are contiguous per channel. The output is stored as
bf16 and upcast to fp32 on the host.

Pipeline per 8-row tile (software-pipelined so PE never waits on ACT):
  DMA : 3 y-block loads (3 tap groups, both halves stacked)
  DVE : per tap group, ONE tensor_tensor multiplies all 3 taps in place
        over the y slot (overlapping-window AP on x, bf16 2x mode); tap
        groups 1,2 fold their last two products so 7 tensors reach PE
  PE  : 7 identity matmuls accumulate taps in PSUM fp32 (bf16 in,
        512-free chunks, ps1 double-buffered)
  ACT : PSUM -> padded bf16 SBUF out1 tile (+halo rows from prev tile)
  PE  : 3x3 conv = 9 block-diagonal bf16 matmuls into PSUM per 2 rows
        (emitted AFTER the next tile's tap matmuls)
  ACT : Prelu(conv + bias, alpha=0.2) -> SBUF bf16, DMA out
"""
import sys

if "/opt/trn_rl_repo" not in sys.path:
    sys.path.append("/opt/trn_rl_repo")

import json

import ml_dtypes
import numpy as np

import concourse.bass as bass
import concourse.tile as tile
from concourse import mybir
from concourse.bass_utils import run_bass_kernel_spmd

F32 = mybir.dt.float32
F32R = mybir.dt.float32r
BF16 = mybir.dt.bfloat16
NP_BF16 = ml_dtypes.bfloat16

N, C, H, W = 4, 64, 192, 192
DIL = 2
N_CORES = 8
HS = H // 2          # rows per core (96)
HH = HS // 2         # rows per half (48)
OUT2 = [6, 8, 8, 8, 8, 8, 2]     # out rows per tile (sum = 48)
T = len(OUT2)
A = [sum(OUT2[:i]) for i in range(T + 1)]  # tile start rows
RMAX = max(OUT2) + 2


def _tile_rows(t):
    # out1 rows computed in tile t
    return OUT2[0] + 2 if t == 0 else OUT2[t]


def _y_block_offsets():
    """(t, g) -> offset into the flat y-prep array (128-partition blocks)."""
    offs = {}
    off = 0
    for t in range(T):
        rows = _tile_rows(t)
        for g in range(3):
            offs[(t, g)] = off
            off += 2 * C * 3 * rows * W
    return offs, off


_Y_OFFS, _Y_TOTAL = _y_block_offsets()


def _legalize_waits(nc):
    """This container's walrus accepts at most ONE sync wait per instruction.
    Split any instruction with k>1 waits into k-1 single-wait NoOps inserted
    immediately before it on the same engine."""
    raw = json.loads(type(nc).to_json_bytes(nc))
    counter = [0]
    for func in raw.get("functions", []):
        for blk in func.get("blocks", []):
            new_insts = []
            for inst in blk.get("instructions", []):
                si = inst.get("sync_info")
                waits = (si or {}).get("on_wait") or []
                if len(waits) > 1:
                    for w in waits[:-1]:
                        counter[0] += 1
                        new_insts.append(
                            {
                                "engine": inst["engine"],
                                "ins": [],
                                "name": f"wsplit_{counter[0]}",
                                "opcode": "NoOp",
                                "outs": [],
                                "sync_info": {"on_update": [], "on_wait": [w]},
                            }
                        )
                    si["on_wait"] = [waits[-1]]
                new_insts.append(inst)
            blk["instructions"] = new_insts
    fixed = json.dumps(raw).encode()
    nc.to_json_bytes = lambda: fixed


def build_nc():
    nc = bass.Bass()
    xin = nc.declare_dram_parameter("xin", [2 * C, HH + 6, W + 4], BF16, isOutput=False)
    yp = nc.declare_dram_parameter("yp", [_Y_TOTAL], BF16, isOutput=False)
    w9 = nc.declare_dram_parameter("w9", [128, 9, 128], BF16, isOutput=False)
    ident = nc.declare_dram_parameter("ident", [128, 128], BF16, isOutput=False)
    bias = nc.declare_dram_parameter("bias", [128, 1], F32, isOutput=False)
    out = nc.declare_dram_parameter("out", [2 * C, HH, W], BF16, isOutput=True)

    XROWS = HH + 6  # x rows per half (54)

    with tile.TileContext(nc) as tc:
        with (
            tc.tile_pool(name="consts", bufs=1) as consts,
            tc.tile_pool(name="ypool", bufs=9) as ypool,
            tc.tile_pool(name="out2p", bufs=2) as out2p,
            tc.tile_pool(name="ps1", bufs=2, space="PSUM") as ps1,
            tc.tile_pool(name="ps2", bufs=2, space="PSUM") as ps2,
        ):
            # small consts first so nothing downstream queues behind big loads
            xp = consts.tile([128, XROWS, W + 4], BF16)
            nc.sync.dma_start(xp[:, 0:10, :], xin[:, 0:10, :])
            id_sb = consts.tile([128, 128], BF16)
            nc.sync.dma_start(id_sb[:], ident[:])
            b_sb = consts.tile([128, 1], F32)
            nc.sync.dma_start(b_sb[:], bias[:])
            nc.sync.dma_start(xp[:, 10:22, :], xin[:, 10:22, :])
            w_sb = consts.tile([128, 9, 128], BF16)
            nc.sync.dma_start(w_sb[:], w9[:])
            nc.sync.dma_start(xp[:, 22:XROWS, :], xin[:, 22:XROWS, :])

            # two persistent padded out1 tiles, pad columns zeroed once
            o1a = consts.tile([128, RMAX, W + 2], BF16)
            o1b = consts.tile([128, RMAX, W + 2], BF16)
            o1_pair = [o1a, o1b]
            for o1x in o1_pair:
                nc.vector.memset(o1x[:, :, 0:1], 0)
                nc.vector.memset(o1x[:, :, W + 1 : W + 2], 0)

            pe_slots = [(0, 0), (0, 1), (0, 2), (1, 0), (1, 1), (2, 0), (2, 1)]

            def emit_products(t, ysl):
                rows = _tile_rows(t)
                for g in range(3):
                    x0 = (2 * g) if t == 0 else (A[t] + 2 * g + 2)
                    for kk in range(3):
                        pv = ysl[g][:, kk, :, :]
                        nc.vector.tensor_tensor(
                            pv,
                            xp[:, x0 : x0 + rows, 2 * kk : 2 * kk + W],
                            pv,
                            op=mybir.AluOpType.mult,
                        )
                    if g > 0:
                        nc.vector.tensor_tensor(
                            ysl[g][:, 1, :, :], ysl[g][:, 1, :, :],
                            ysl[g][:, 2, :, :], op=mybir.AluOpType.add,
                        )

            def emit_taps(t, ysl):
                rows = _tile_rows(t)
                nfree = rows * W
                p1 = ps1.tile([128, nfree], F32, tag="p1")
                for i, (g, kk) in enumerate(pe_slots):
                    prf = ysl[g][:, kk, :, :].rearrange("p r w -> p (r w)")
                    for c0 in range(0, nfree, 512):
                        cn = min(512, nfree - c0)
                        nc.tensor.matmul(
                            p1[:, c0 : c0 + cn],
                            id_sb[:],
                            prf[:, c0 : c0 + cn],
                            start=(i == 0),
                            stop=(i == len(pe_slots) - 1),
                        )
                return p1

            prev_o1 = None
            prev_rows2 = 0
            conv_args = None

            def emit_conv(o1c, r2c, tc_):
                # 3x3 conv in 2-row chunks, 9 block-diagonal matmuls each
                o2 = out2p.tile([128, r2c, W], BF16, tag="out2")
                for j in range(r2c // 2):
                    p2 = ps2.tile([128, 2, W], F32, tag="p2")
                    for tp in range(9):
                        ki, kj = divmod(tp, 3)
                        nc.tensor.matmul(
                            p2[:],
                            w_sb[:, tp, :],
                            o1c[:, 2 * j + ki : 2 * j + ki + 2, kj : kj + W],
                            start=(tp == 0),
                            stop=(tp == 8),
                        )
                    nc.scalar.activation(
                        o2[:, 2 * j : 2 * j + 2, :],
                        p2[:],
                        mybir.ActivationFunctionType.Prelu,
                        bias=b_sb[:, 0:1],
                        scale=1.0,
                        alpha=0.2,
                    )
                nc.sync.dma_start(out[:, A[tc_] : A[tc_] + r2c, :], o2[:])

            # ---- last tile (2 rows): prefetch y into persistent tiles on
            # the sync queue and run its products during the warmup, so the
            # end-of-kernel chain is just taps+copy+conv+store
            tl = T - 1
            lrows = _tile_rows(tl)
            y6a = consts.tile([128, 3, lrows, W], BF16)
            y6b = consts.tile([128, 3, lrows, W], BF16)
            y6c = consts.tile([128, 3, lrows, W], BF16)
            y6l = [y6a, y6b, y6c]

            def emit_y6_loads():
                for g in range(3):
                    off = _Y_OFFS[(tl, g)]
                    blk = 2 * C * 3 * lrows * W
                    src = yp[off : off + blk].rearrange("(c f) -> c f", c=2 * C)
                    nc.gpsimd.dma_start(
                        y6l[g].rearrange("p a b c -> p (a b c)"), src
                    )

            for t in range(T - 1):
                rows = _tile_rows(t)
                r2 = OUT2[t]

                ysl = []
                for g in range(3):
                    ys = ypool.tile([128, 3, rows, W], BF16, tag="y")
                    off = _Y_OFFS[(t, g)]
                    blk = 2 * C * 3 * rows * W
                    src = yp[off : off + blk].rearrange("(c f) -> c f", c=2 * C)
                    dst = ys.rearrange("p a b c -> p (a b c)")
                    nc.gpsimd.dma_start(dst, src)
                    ysl.append(ys)
                if t == 2:
                    emit_y6_loads()
                emit_products(t, ysl)
                if t == 3:
                    emit_products(tl, y6l)

                if t == T - 2 and conv_args is not None:
                    # drain the pipeline skew: don't hold conv(t-1) hostage
                    # to taps(t), whose y data arrives last
                    emit_conv(*conv_args)
                    conv_args = None

                p1 = emit_taps(t, ysl)

                # ---- PSUM -> padded bf16 out1 (+halo from prev tile) ----
                o1 = o1_pair[t % 2]
                r0 = 0 if t == 0 else 2
                p1v = p1.rearrange("p (r w) -> p r w", w=W)
                nc.scalar.copy(o1[:, r0 : r0 + rows, 1 : W + 1], p1v[:])
                if t > 0:
                    nc.scalar.copy(
                        o1[:, 0:2, :],
                        prev_o1[:, prev_rows2 : prev_rows2 + 2, :],
                    )
                prev_o1 = o1
                prev_rows2 = r2

                # ---- software pipeline: conv of tile t-1 after taps of t ----
                if conv_args is not None:
                    emit_conv(*conv_args)
                conv_args = (o1, r2, t)

            # ---- last tile epilogue ----
            p1 = emit_taps(tl, y6l)
            o1 = o1_pair[tl % 2]
            p1v = p1.rearrange("p (r w) -> p r w", w=W)
            nc.scalar.copy(o1[:, 2 : 2 + lrows, 1 : W + 1], p1v[:])
            nc.scalar.copy(
                o1[:, 0:2, :], prev_o1[:, prev_rows2 : prev_rows2 + 2, :]
            )
            emit_conv(*conv_args)
            emit_conv(o1, OUT2[tl], tl)
    _legalize_waits(nc)
    return nc


_NC_CACHE = None


def _get_nc():
    global _NC_CACHE
    if _NC_CACHE is None:
        _NC_CACHE = build_nc()
    return _NC_CACHE


def _bf16(a):
    """Fast vectorized fp32 -> bf16 with round-to-nearest-even."""
    a = np.ascontiguousarray(a, dtype=np.float32)
    b = a.view(np.uint32)
    r = ((b + 0x7FFF + ((b >> 16) & 1)) >> 16).astype(np.uint16)
    return r.view(NP_BF16)


def _f32(a):
    """bf16 -> fp32 upcast."""
    u = np.ascontiguousarray(a).view(np.uint16).astype(np.uint32) << 16
    return u.view(np.float32)


def _prep_core_inputs(x, y, n, s):
    xn = x[n] if x.dtype == NP_BF16 else _bf16(np.asarray(x[n]))
    yn = y[n] if y.dtype == NP_BF16 else _bf16(np.asarray(y[n]))

    h0 = s * HS
    xpad = np.zeros((C, HS + 6, W + 4), dtype=NP_BF16)
    a, b = max(0, h0 - 3), min(H, h0 + HS + 3)
    xpad[:, a - (h0 - 3) : b - (h0 - 3), 2 : W + 2] = xn[:, a:b, :]
    # stacked halves: [2, C, 54, W+4] -> [128, 54, W+4]
    XR = HH + 6
    xin = np.concatenate(
        [xpad[:, 0:XR, :], xpad[:, HH : HH + XR, :]], axis=0
    ).reshape(2 * C, XR, W + 4)

    # padded y rows [h0-1, h0+97), as [C, 9, 98, W]
    yin = np.zeros((C, 9, HS + 2, W), dtype=NP_BF16)
    a, b = max(0, h0 - 1), min(H, h0 + HS + 1)
    yin[:, :, a - (h0 - 1) : b - (h0 - 1), :] = yn.reshape(C, 9, H, W)[
        :, :, a:b, :
    ]

    ypf = np.empty(_Y_TOTAL, dtype=NP_BF16)
    for t in range(T):
        rows = _tile_rows(t)
        r0 = 0 if t == 0 else A[t] + 2
        for g in range(3):
            off = _Y_OFFS[(t, g)]
            blk = C * 3 * rows * W
            for half in range(2):
                rr = r0 + HH * half
                ypf[off + half * blk : off + (half + 1) * blk] = yin[
                    :, 3 * g : 3 * g + 3, rr : rr + rows, :
                ].reshape(-1)
    return xin, ypf


def _prep_weights(fuse_w, fuse_b):
    # pre-permuted [in_ch_partition, tap, out_ch] so the SBUF load is one
    # contiguous run per partition
    w9 = np.zeros((128, 9, 128), dtype=NP_BF16)
    for tp in range(9):
        ki, kj = divmod(tp, 3)
        wt = fuse_w[:, :, ki, kj].T.astype(NP_BF16)  # [i, o]
        w9[0:64, tp, 0:64] = wt
        w9[64:128, tp, 64:128] = wt
    ident = np.eye(128, dtype=NP_BF16)
    bias = np.concatenate([fuse_b, fuse_b]).reshape(128, 1).astype(np.float32)
    return w9, ident, bias


def kernel(x, y, fuse_w, fuse_b):
    x = _bf16(np.asarray(x, dtype=np.float32))
    y = _bf16(np.asarray(y, dtype=np.float32))
    fuse_w = np.asarray(fuse_w, dtype=np.float32)
    fuse_b = np.asarray(fuse_b, dtype=np.float32)

    w9, ident, bias = _prep_weights(fuse_w, fuse_b)

    in_maps = []
    for c in range(N_CORES):
        n, s = divmod(c, 2)
        xin, ypf = _prep_core_inputs(x, y, n, s)
        in_maps.append(
            {
                "xin": xin,
                "yp": ypf,
                "w9": w9,
                "ident": ident,
                "bias": bias,
            }
        )

    nc = _get_nc()
    res = run_bass_kernel_spmd(nc, in_maps, list(range(N_CORES)))

    full = np.empty((N, C, H, W), dtype=np.float32)
    for c in range(N_CORES):
        n, s = divmod(c, 2)
        o4 = _f32(res.results[c]["out"]).reshape(2, C, HH, W)
        for half in range(2):
            r = s * HS + half * HH
            full[n, :, r : r + HH, :] = o4[half]
    return full
